# revision 4
# baseline (speedup 1.0000x reference)
"""Trainium2 Bass kernel for nn_CropAndPadMask (paste instance masks into canvases).

Math: for each (b, n) box the reference output is a bilinear resize of the
28x28 mask pasted into a zero [H, W] canvas.  Bilinear resize + paste is a
pair of small matmuls:

    out[b, n] = Wy[b, n] @ mask[b, n] @ Wx[b, n]

with Wy [H, 28] holding the y-interpolation weights (box/validity mask folded
in) and Wx [28, W] the x-interpolation weights.  The host precomputes these
tiny weight matrices from det_outs; the device does the heavy part: 5 matmuls
per canvas on TensorE and streams the 256 MiB of canvases out to HBM.

Sharding: 256 canvases are split 32-per-core across the 8 NeuronCores
(pure data parallel, no communication).
"""

import sys

for _p in ("/opt/trn_rl_repo", "/root/.axon_site/_ro/trn_rl_repo"):
    if _p not in sys.path:
        sys.path.append(_p)

import numpy as np

import concourse.bass as bass
import concourse.mybir as mybir
import concourse.tile as tile
from concourse.bass_utils import run_bass_kernel_spmd

B, N, H, W, MH, MW = 4, 64, 512, 512, 28, 28
N_CORES = 8
CPC = (B * N) // N_CORES  # canvases per core = 32
G = 4                     # canvases per output DMA group
KCH = 4                   # row chunks per canvas (H = KCH * 128)
FREE = 2 * W + MW         # per-canvas weight row: wyT | wx | maskT

def _split_multi_waits(nc: bass.Bass) -> None:
    """The walrus bundled in this container accepts at most ONE sync wait per
    instruction.  Tile freely attaches several.  Hoist the extras onto
    standalone EventSemaphore carriers inserted just before the instruction on
    the same engine (per-engine program order makes this equivalent)."""
    n_new = 0
    for f in nc.m.functions:
        for bb in f.blocks:
            lst = bb.instructions
            i = 0
            while i < len(lst):
                ins = lst[i]
                si = ins.sync_info
                if si is not None and si.on_wait and len(si.on_wait) > 1:
                    waits = list(si.on_wait)
                    ins.sync_info = mybir.SyncInfo(
                        on_wait=waits[:1], on_update=list(si.on_update or [])
                    )
                    carriers = []
                    for w in waits[1:]:
                        n_new += 1
                        carriers.append(
                            mybir.InstEventSemaphore(
                                name=f"I-waitsplit-{n_new}",
                                ins=[],
                                outs=[],
                                engine=ins.engine,
                                sync_info=mybir.SyncInfo(on_wait=[w], on_update=[]),
                            )
                        )
                    lst[i:i] = carriers
                    i += len(carriers)
                i += 1


def build_nc(cpc: int = CPC, g: int = G) -> bass.Bass:
    """One core's program: cpc canvases, streamed out g at a time."""
    f32 = mybir.dt.float32
    nc = bass.Bass()
    wmat = nc.dram_tensor("wmat", [cpc, MW, FREE], f32, kind="ExternalInput")
    out = nc.dram_tensor("out", [cpc, H, W], f32, kind="ExternalOutput")

    with tile.TileContext(nc) as tc:
        with (
            tc.tile_pool(name="win", bufs=3) as win_pool,
            tc.tile_pool(name="ssb", bufs=3) as s_pool,
            tc.tile_pool(name="stage", bufs=2) as stage_pool,
            tc.tile_pool(name="psum_s", bufs=2, space="PSUM") as psum_s,
            tc.tile_pool(name="psum_c", bufs=4, space="PSUM") as psum_c,
        ):
            for grp in range(cpc // g):
                stage = stage_pool.tile([128, g, KCH, W], f32)
                for cc in range(g):
                    c = grp * g + cc
                    w_t = win_pool.tile([MW, FREE], f32)
                    nc.gpsimd.dma_start(w_t[:], wmat[c])
                    # S = mask @ Wx : lhsT = mask^T [MW, MH], rhs = Wx [MW, W]
                    s_p = psum_s.tile([MH, W], f32)
                    nc.tensor.matmul(
                        s_p[:], w_t[:, 2 * W :], w_t[:, W : 2 * W],
                        start=True, stop=True,
                    )
                    s_sb = s_pool.tile([MH, W], f32)
                    nc.vector.tensor_copy(s_sb[:], s_p[:])
                    for k in range(KCH):
                        # rows {k, k+4, ...} of the canvas: lhsT = WyT cols k::4
                        p_k = psum_c.tile([128, W], f32)
                        nc.tensor.matmul(
                            p_k[:], w_t[:, 128 * k : 128 * (k + 1)], s_sb[:],
                            start=True, stop=True,
                        )
                        if k % 2 == 0:
                            nc.scalar.copy(stage[:, cc, k, :], p_k[:])
                        else:
                            nc.vector.tensor_copy(stage[:, cc, k, :], p_k[:])
                # canvas row = 4*p + k  ->  DRAM view [p, c, k, w]
                out_ap = out[grp * g : (grp + 1) * g].rearrange(
                    "c (p k) w -> p c k w", k=KCH
                )
                nc.sync.dma_start(out_ap, stage[:])
    _split_multi_waits(nc)
    return nc


def _box_weight_matrices(det_outs: np.ndarray) -> tuple[np.ndarray, np.ndarray]:
    """Wy [BN, H, MH] and Wx [BN, MW, W] (f32), reference semantics."""
    det = np.asarray(det_outs, dtype=np.float32).reshape(B * N, 6)
    score = det[:, 5]
    thr = np.float32(50.0) if np.max(score) > 50.0 else np.float32(-100.0)
    valid = score >= thr
    box = np.maximum(det, np.float32(1.0))
    cx, cy, w, h = box[:, 0], box[:, 1], box[:, 2], box[:, 3]
    two = np.float32(2.0)
    xmin = np.clip(np.ceil(cx - w / two).astype(np.int32), 0, W)
    xmax = np.clip(np.ceil(cx + w / two).astype(np.int32), 0, W)
    ymin = np.clip(np.ceil(cy - h / two).astype(np.int32), 0, H)
    ymax = np.clip(np.ceil(cy + h / two).astype(np.int32), 0, H)
    out_h = (ymax - ymin).astype(np.float32)
    out_w = (xmax - xmin).astype(np.float32)
    one = np.float32(1.0)
    sy = np.where(out_h > one, np.float32(MH - 1) / np.maximum(out_h - one, one),
                  np.float32(0.0)).astype(np.float32)
    sx = np.where(out_w > one, np.float32(MW - 1) / np.maximum(out_w - one, one),
                  np.float32(0.0)).astype(np.float32)

    ys = np.arange(H, dtype=np.float32)
    xs = np.arange(W, dtype=np.float32)
    src_y = (ys[None, :] - ymin[:, None].astype(np.float32)) * sy[:, None]
    src_x = (xs[None, :] - xmin[:, None].astype(np.float32)) * sx[:, None]
    src_y = np.clip(src_y, np.float32(0.0), np.float32(MH - 1)).astype(np.float32)
    src_x = np.clip(src_x, np.float32(0.0), np.float32(MW - 1)).astype(np.float32)

    y0 = np.floor(src_y).astype(np.int32)
    y1 = np.minimum(y0 + 1, MH - 1)
    wy = (src_y - y0.astype(np.float32)).astype(np.float32)
    x0 = np.floor(src_x).astype(np.int32)
    x1 = np.minimum(x0 + 1, MW - 1)
    wx = (src_x - x0.astype(np.float32)).astype(np.float32)

    keep_y = ((ys[None, :] >= ymin[:, None].astype(np.float32))
              & (ys[None, :] < ymax[:, None].astype(np.float32))
              & valid[:, None]).astype(np.float32)
    keep_x = ((xs[None, :] >= xmin[:, None].astype(np.float32))
              & (xs[None, :] < xmax[:, None].astype(np.float32))).astype(np.float32)

    m = np.arange(MH, dtype=np.int32)
    Wy = ((m[None, None, :] == y0[:, :, None]) * (one - wy[:, :, None])
          + (m[None, None, :] == y1[:, :, None]) * wy[:, :, None]).astype(np.float32)
    Wy *= keep_y[:, :, None]
    Wx = ((m[None, :, None] == x0[:, None, :]) * (one - wx[:, None, :])
          + (m[None, :, None] == x1[:, None, :]) * wx[:, None, :]).astype(np.float32)
    Wx *= keep_x[:, None, :]
    return Wy, Wx


_ROW_PERM = np.concatenate([KCH * np.arange(H // KCH) + k for k in range(KCH)])


def prepare_in_maps(det_outs: np.ndarray, ins_outs: np.ndarray,
                    cpc: int = CPC, n_cores: int = N_CORES) -> list[dict]:
    Wy, Wx = _box_weight_matrices(det_outs)
    # wyT [BN, MH, H], columns permuted so block k holds rows k::KCH
    wyT = np.ascontiguousarray(np.transpose(Wy, (0, 2, 1)))[:, :, _ROW_PERM]
    masksT = np.ascontiguousarray(
        np.transpose(np.asarray(ins_outs, np.float32).reshape(B * N, MH, MW),
                     (0, 2, 1)))
    wmat = np.concatenate([wyT, Wx, masksT], axis=2).astype(np.float32)
    assert wmat.shape == (B * N, MW, FREE)
    return [{"wmat": np.ascontiguousarray(wmat[i * cpc : (i + 1) * cpc])}
            for i in range(n_cores)]


def kernel(images: np.ndarray, det_outs: np.ndarray, ins_outs: np.ndarray) -> np.ndarray:
    nc = build_nc()
    in_maps = prepare_in_maps(det_outs, ins_outs)
    res = run_bass_kernel_spmd(nc, in_maps, list(range(N_CORES)))
    full = np.concatenate([res.results[i]["out"] for i in range(N_CORES)], axis=0)
    return full.reshape(B, N, H, W).astype(np.float32)


# revision 5
# speedup vs baseline: 1.8117x; 1.8117x over previous
"""Trainium2 Bass kernel for nn_CropAndPadMask (paste instance masks into canvases).

Math: for each (b, n) box the reference output is a bilinear resize of the
28x28 mask pasted into a zero [H, W] canvas.  Bilinear resize + paste is a
pair of small matmuls:

    out[b, n] = Wy[b, n] @ mask[b, n] @ Wx[b, n]

with Wy [H, 28] holding the y-interpolation weights (box/validity mask folded
in) and Wx [28, W] the x-interpolation weights.  The host precomputes these
tiny weight matrices from det_outs; the device does the heavy part: 5 matmuls
per canvas on TensorE and streams the 256 MiB of canvases out to HBM.

Sharding: 256 canvases are split 32-per-core across the 8 NeuronCores
(pure data parallel, no communication).
"""

import sys

for _p in ("/opt/trn_rl_repo", "/root/.axon_site/_ro/trn_rl_repo"):
    if _p not in sys.path:
        sys.path.append(_p)

import numpy as np

import concourse.bass as bass
import concourse.mybir as mybir
import concourse.tile as tile
from concourse.bass_utils import run_bass_kernel_spmd

B, N, H, W, MH, MW = 4, 64, 512, 512, 28, 28
N_CORES = 8
CPC = (B * N) // N_CORES  # canvases per core = 32
G = 4                     # canvases per output DMA group
KCH = 4                   # row chunks per canvas (H = KCH * 128)
FREE = 2 * W + MW         # per-canvas weight row: wyT | wx | maskT

def _split_multi_waits(nc: bass.Bass) -> None:
    """The walrus bundled in this container accepts at most ONE sync wait per
    instruction.  Tile freely attaches several.  Hoist the extras onto
    standalone EventSemaphore carriers inserted just before the instruction on
    the same engine (per-engine program order makes this equivalent)."""
    n_new = 0
    for f in nc.m.functions:
        for bb in f.blocks:
            lst = bb.instructions
            i = 0
            while i < len(lst):
                ins = lst[i]
                si = ins.sync_info
                if si is not None and si.on_wait and len(si.on_wait) > 1:
                    waits = list(si.on_wait)
                    ins.sync_info = mybir.SyncInfo(
                        on_wait=waits[:1], on_update=list(si.on_update or [])
                    )
                    carriers = []
                    for w in waits[1:]:
                        n_new += 1
                        carriers.append(
                            mybir.InstEventSemaphore(
                                name=f"I-waitsplit-{n_new}",
                                ins=[],
                                outs=[],
                                engine=ins.engine,
                                sync_info=mybir.SyncInfo(on_wait=[w], on_update=[]),
                            )
                        )
                    lst[i:i] = carriers
                    i += len(carriers)
                i += 1


def build_nc(cpc: int = CPC, g: int = G) -> bass.Bass:
    """One core's program: cpc canvases, streamed out g at a time."""
    f32 = mybir.dt.float32
    f32r = mybir.dt.float32r
    nc = bass.Bass()
    wmat = nc.dram_tensor("wmat", [cpc, MW, FREE], f32r, kind="ExternalInput")
    out = nc.dram_tensor("out", [cpc, H, W], f32, kind="ExternalOutput")

    with tile.TileContext(nc) as tc:
        with (
            tc.tile_pool(name="win", bufs=3) as win_pool,
            tc.tile_pool(name="ssb", bufs=3) as s_pool,
            tc.tile_pool(name="stage", bufs=2) as stage_pool,
            tc.tile_pool(name="psum_s", bufs=2, space="PSUM") as psum_s,
            tc.tile_pool(name="psum_c", bufs=4, space="PSUM") as psum_c,
        ):
            for grp in range(cpc // g):
                stage = stage_pool.tile([128, g, KCH, W], f32)
                for cc in range(g):
                    c = grp * g + cc
                    w_t = win_pool.tile([MW, FREE], f32r)
                    nc.gpsimd.dma_start(w_t[:], wmat[c])
                    # S = mask @ Wx : lhsT = mask^T [MW, MH], rhs = Wx [MW, W]
                    s_p = psum_s.tile([MH, W], f32)
                    nc.tensor.matmul(
                        s_p[:], w_t[:, 2 * W :], w_t[:, W : 2 * W],
                        start=True, stop=True,
                    )
                    s_sb = s_pool.tile([MH, W], f32r)
                    nc.vector.tensor_copy(s_sb[:], s_p[:])
                    for k in range(KCH):
                        # rows {k, k+4, ...} of the canvas: lhsT = WyT cols k::4
                        p_k = psum_c.tile([128, W], f32)
                        nc.tensor.matmul(
                            p_k[:], w_t[:, 128 * k : 128 * (k + 1)], s_sb[:],
                            start=True, stop=True,
                        )
                        if k % 2 == 0:
                            nc.scalar.copy(stage[:, cc, k, :], p_k[:])
                        else:
                            nc.vector.tensor_copy(stage[:, cc, k, :], p_k[:])
                # canvas row = 4*p + k  ->  DRAM view [p, c, k, w]
                out_ap = out[grp * g : (grp + 1) * g].rearrange(
                    "c (p k) w -> p c k w", k=KCH
                )
                nc.sync.dma_start(out_ap, stage[:])
    _split_multi_waits(nc)
    return nc


def _box_weight_matrices(det_outs: np.ndarray) -> tuple[np.ndarray, np.ndarray]:
    """Wy [BN, H, MH] and Wx [BN, MW, W] (f32), reference semantics."""
    det = np.asarray(det_outs, dtype=np.float32).reshape(B * N, 6)
    score = det[:, 5]
    thr = np.float32(50.0) if np.max(score) > 50.0 else np.float32(-100.0)
    valid = score >= thr
    box = np.maximum(det, np.float32(1.0))
    cx, cy, w, h = box[:, 0], box[:, 1], box[:, 2], box[:, 3]
    two = np.float32(2.0)
    xmin = np.clip(np.ceil(cx - w / two).astype(np.int32), 0, W)
    xmax = np.clip(np.ceil(cx + w / two).astype(np.int32), 0, W)
    ymin = np.clip(np.ceil(cy - h / two).astype(np.int32), 0, H)
    ymax = np.clip(np.ceil(cy + h / two).astype(np.int32), 0, H)
    out_h = (ymax - ymin).astype(np.float32)
    out_w = (xmax - xmin).astype(np.float32)
    one = np.float32(1.0)
    sy = np.where(out_h > one, np.float32(MH - 1) / np.maximum(out_h - one, one),
                  np.float32(0.0)).astype(np.float32)
    sx = np.where(out_w > one, np.float32(MW - 1) / np.maximum(out_w - one, one),
                  np.float32(0.0)).astype(np.float32)

    ys = np.arange(H, dtype=np.float32)
    xs = np.arange(W, dtype=np.float32)
    src_y = (ys[None, :] - ymin[:, None].astype(np.float32)) * sy[:, None]
    src_x = (xs[None, :] - xmin[:, None].astype(np.float32)) * sx[:, None]
    src_y = np.clip(src_y, np.float32(0.0), np.float32(MH - 1)).astype(np.float32)
    src_x = np.clip(src_x, np.float32(0.0), np.float32(MW - 1)).astype(np.float32)

    y0 = np.floor(src_y).astype(np.int32)
    y1 = np.minimum(y0 + 1, MH - 1)
    wy = (src_y - y0.astype(np.float32)).astype(np.float32)
    x0 = np.floor(src_x).astype(np.int32)
    x1 = np.minimum(x0 + 1, MW - 1)
    wx = (src_x - x0.astype(np.float32)).astype(np.float32)

    keep_y = ((ys[None, :] >= ymin[:, None].astype(np.float32))
              & (ys[None, :] < ymax[:, None].astype(np.float32))
              & valid[:, None]).astype(np.float32)
    keep_x = ((xs[None, :] >= xmin[:, None].astype(np.float32))
              & (xs[None, :] < xmax[:, None].astype(np.float32))).astype(np.float32)

    m = np.arange(MH, dtype=np.int32)
    Wy = ((m[None, None, :] == y0[:, :, None]) * (one - wy[:, :, None])
          + (m[None, None, :] == y1[:, :, None]) * wy[:, :, None]).astype(np.float32)
    Wy *= keep_y[:, :, None]
    Wx = ((m[None, :, None] == x0[:, None, :]) * (one - wx[:, None, :])
          + (m[None, :, None] == x1[:, None, :]) * wx[:, None, :]).astype(np.float32)
    Wx *= keep_x[:, None, :]
    return Wy, Wx


_ROW_PERM = np.concatenate([KCH * np.arange(H // KCH) + k for k in range(KCH)])


def prepare_in_maps(det_outs: np.ndarray, ins_outs: np.ndarray,
                    cpc: int = CPC, n_cores: int = N_CORES) -> list[dict]:
    Wy, Wx = _box_weight_matrices(det_outs)
    # wyT [BN, MH, H], columns permuted so block k holds rows k::KCH
    wyT = np.ascontiguousarray(np.transpose(Wy, (0, 2, 1)))[:, :, _ROW_PERM]
    masksT = np.ascontiguousarray(
        np.transpose(np.asarray(ins_outs, np.float32).reshape(B * N, MH, MW),
                     (0, 2, 1)))
    wmat = np.concatenate([wyT, Wx, masksT], axis=2).astype(np.float32)
    assert wmat.shape == (B * N, MW, FREE)
    return [{"wmat": np.ascontiguousarray(wmat[i * cpc : (i + 1) * cpc])}
            for i in range(n_cores)]


def kernel(images: np.ndarray, det_outs: np.ndarray, ins_outs: np.ndarray) -> np.ndarray:
    nc = build_nc()
    in_maps = prepare_in_maps(det_outs, ins_outs)
    res = run_bass_kernel_spmd(nc, in_maps, list(range(N_CORES)))
    full = np.concatenate([res.results[i]["out"] for i in range(N_CORES)], axis=0)
    return full.reshape(B, N, H, W).astype(np.float32)


# revision 10
# speedup vs baseline: 1.9808x; 1.0934x over previous
"""Trainium2 Bass kernel for nn_CropAndPadMask (paste instance masks into canvases).

Math: for each (b, n) box the reference output is a bilinear resize of the
28x28 mask pasted into a zero [H, W] canvas.  Bilinear resize + paste is a
pair of small matmuls:

    out[b, n] = Wy[b, n] @ mask[b, n] @ Wx[b, n]

with Wy [H, 28] holding the y-interpolation weights (box/validity mask folded
in) and Wx [28, W] the x-interpolation weights.  The host precomputes these
tiny weight matrices from det_outs; the device does the heavy part: 5 matmuls
per canvas on TensorE and streams the 256 MiB of canvases out to HBM.

Sharding: 256 canvases are split 32-per-core across the 8 NeuronCores
(pure data parallel, no communication).
"""

import sys

for _p in ("/opt/trn_rl_repo", "/root/.axon_site/_ro/trn_rl_repo"):
    if _p not in sys.path:
        sys.path.append(_p)

import numpy as np

import concourse.bass as bass
import concourse.mybir as mybir
import concourse.tile as tile
from concourse.bass_utils import run_bass_kernel_spmd

B, N, H, W, MH, MW = 4, 64, 512, 512, 28, 28
N_CORES = 8
CPC = (B * N) // N_CORES  # canvases per core = 32
G = 2                     # canvases per output DMA group
KCH = 4                   # row chunks per canvas (H = KCH * 128)
FREE = 2 * W + MW         # per-canvas weight row: wyT | wx | maskT

def _split_multi_waits(nc: bass.Bass) -> None:
    """The walrus bundled in this container accepts at most ONE sync wait per
    instruction.  Tile freely attaches several.  Hoist the extras onto
    standalone EventSemaphore carriers inserted just before the instruction on
    the same engine (per-engine program order makes this equivalent)."""
    n_new = 0
    for f in nc.m.functions:
        for bb in f.blocks:
            lst = bb.instructions
            i = 0
            while i < len(lst):
                ins = lst[i]
                si = ins.sync_info
                if si is not None and si.on_wait and len(si.on_wait) > 1:
                    waits = list(si.on_wait)
                    ins.sync_info = mybir.SyncInfo(
                        on_wait=waits[:1], on_update=list(si.on_update or [])
                    )
                    carriers = []
                    for w in waits[1:]:
                        n_new += 1
                        carriers.append(
                            mybir.InstEventSemaphore(
                                name=f"I-waitsplit-{n_new}",
                                ins=[],
                                outs=[],
                                engine=ins.engine,
                                sync_info=mybir.SyncInfo(on_wait=[w], on_update=[]),
                            )
                        )
                    lst[i:i] = carriers
                    i += len(carriers)
                i += 1


def build_nc(cpc: int = CPC, g: int = G) -> bass.Bass:
    """One core's program: cpc canvases, streamed out g at a time."""
    f32 = mybir.dt.float32
    f32r = mybir.dt.float32r
    nc = bass.Bass()
    wmat = nc.dram_tensor("wmat", [cpc, MW, FREE], f32r, kind="ExternalInput")
    out = nc.dram_tensor("out", [cpc, H, W], f32, kind="ExternalOutput")

    with tile.TileContext(nc) as tc:
        with (
            tc.tile_pool(name="win", bufs=4) as win_pool,
            tc.tile_pool(name="ssb", bufs=3) as s_pool,
            tc.tile_pool(name="stage", bufs=4) as stage_pool,
            tc.tile_pool(name="psum_s", bufs=2, space="PSUM") as psum_s,
            tc.tile_pool(name="psum_c", bufs=6, space="PSUM") as psum_c,
        ):
            n_cp = 0
            for grp in range(cpc // g):
                stage = stage_pool.tile([128, g, KCH, W], f32)
                for cc in range(g):
                    c = grp * g + cc
                    w_t = win_pool.tile([MW, FREE], f32r)
                    nc.gpsimd.dma_start(w_t[:], wmat[c])
                    # S = mask @ Wx : lhsT = mask^T [MW, MH], rhs = Wx [MW, W]
                    s_p = psum_s.tile([MH, W], f32)
                    nc.tensor.matmul(
                        s_p[:], w_t[:, 2 * W :], w_t[:, W : 2 * W],
                        start=True, stop=True,
                    )
                    s_sb = s_pool.tile([MH, W], f32r)
                    if n_cp % 2 == 0:
                        nc.scalar.copy(s_sb[:], s_p[:])
                    else:
                        nc.vector.tensor_copy(s_sb[:], s_p[:])
                    n_cp += 1
                    for k in range(KCH):
                        # rows {k, k+4, ...} of the canvas: lhsT = WyT cols k::4
                        p_k = psum_c.tile([128, W], f32)
                        nc.tensor.matmul(
                            p_k[:], w_t[:, 128 * k : 128 * (k + 1)], s_sb[:],
                            start=True, stop=True,
                        )
                        if n_cp % 2 == 0:
                            nc.scalar.copy(stage[:, cc, k, :], p_k[:])
                        else:
                            nc.vector.tensor_copy(stage[:, cc, k, :], p_k[:])
                        n_cp += 1
                # canvas row = 4*p + k  ->  DRAM view [p, c, k, w]
                out_ap = out[grp * g : (grp + 1) * g].rearrange(
                    "c (p k) w -> p c k w", k=KCH
                )
                nc.sync.dma_start(out_ap, stage[:])
    _split_multi_waits(nc)
    return nc


def _box_weight_matrices(det_outs: np.ndarray) -> tuple[np.ndarray, np.ndarray]:
    """Wy [BN, H, MH] and Wx [BN, MW, W] (f32), reference semantics."""
    det = np.asarray(det_outs, dtype=np.float32).reshape(B * N, 6)
    score = det[:, 5]
    thr = np.float32(50.0) if np.max(score) > 50.0 else np.float32(-100.0)
    valid = score >= thr
    box = np.maximum(det, np.float32(1.0))
    cx, cy, w, h = box[:, 0], box[:, 1], box[:, 2], box[:, 3]
    two = np.float32(2.0)
    xmin = np.clip(np.ceil(cx - w / two).astype(np.int32), 0, W)
    xmax = np.clip(np.ceil(cx + w / two).astype(np.int32), 0, W)
    ymin = np.clip(np.ceil(cy - h / two).astype(np.int32), 0, H)
    ymax = np.clip(np.ceil(cy + h / two).astype(np.int32), 0, H)
    out_h = (ymax - ymin).astype(np.float32)
    out_w = (xmax - xmin).astype(np.float32)
    one = np.float32(1.0)
    sy = np.where(out_h > one, np.float32(MH - 1) / np.maximum(out_h - one, one),
                  np.float32(0.0)).astype(np.float32)
    sx = np.where(out_w > one, np.float32(MW - 1) / np.maximum(out_w - one, one),
                  np.float32(0.0)).astype(np.float32)

    ys = np.arange(H, dtype=np.float32)
    xs = np.arange(W, dtype=np.float32)
    src_y = (ys[None, :] - ymin[:, None].astype(np.float32)) * sy[:, None]
    src_x = (xs[None, :] - xmin[:, None].astype(np.float32)) * sx[:, None]
    src_y = np.clip(src_y, np.float32(0.0), np.float32(MH - 1)).astype(np.float32)
    src_x = np.clip(src_x, np.float32(0.0), np.float32(MW - 1)).astype(np.float32)

    y0 = np.floor(src_y).astype(np.int32)
    y1 = np.minimum(y0 + 1, MH - 1)
    wy = (src_y - y0.astype(np.float32)).astype(np.float32)
    x0 = np.floor(src_x).astype(np.int32)
    x1 = np.minimum(x0 + 1, MW - 1)
    wx = (src_x - x0.astype(np.float32)).astype(np.float32)

    keep_y = ((ys[None, :] >= ymin[:, None].astype(np.float32))
              & (ys[None, :] < ymax[:, None].astype(np.float32))
              & valid[:, None]).astype(np.float32)
    keep_x = ((xs[None, :] >= xmin[:, None].astype(np.float32))
              & (xs[None, :] < xmax[:, None].astype(np.float32))).astype(np.float32)

    m = np.arange(MH, dtype=np.int32)
    Wy = ((m[None, None, :] == y0[:, :, None]) * (one - wy[:, :, None])
          + (m[None, None, :] == y1[:, :, None]) * wy[:, :, None]).astype(np.float32)
    Wy *= keep_y[:, :, None]
    Wx = ((m[None, :, None] == x0[:, None, :]) * (one - wx[:, None, :])
          + (m[None, :, None] == x1[:, None, :]) * wx[:, None, :]).astype(np.float32)
    Wx *= keep_x[:, None, :]
    return Wy, Wx


_ROW_PERM = np.concatenate([KCH * np.arange(H // KCH) + k for k in range(KCH)])


def prepare_in_maps(det_outs: np.ndarray, ins_outs: np.ndarray,
                    cpc: int = CPC, n_cores: int = N_CORES) -> list[dict]:
    Wy, Wx = _box_weight_matrices(det_outs)
    # wyT [BN, MH, H], columns permuted so block k holds rows k::KCH
    wyT = np.ascontiguousarray(np.transpose(Wy, (0, 2, 1)))[:, :, _ROW_PERM]
    masksT = np.ascontiguousarray(
        np.transpose(np.asarray(ins_outs, np.float32).reshape(B * N, MH, MW),
                     (0, 2, 1)))
    wmat = np.concatenate([wyT, Wx, masksT], axis=2).astype(np.float32)
    assert wmat.shape == (B * N, MW, FREE)
    return [{"wmat": np.ascontiguousarray(wmat[i * cpc : (i + 1) * cpc])}
            for i in range(n_cores)]


def kernel(images: np.ndarray, det_outs: np.ndarray, ins_outs: np.ndarray) -> np.ndarray:
    nc = build_nc()
    in_maps = prepare_in_maps(det_outs, ins_outs)
    res = run_bass_kernel_spmd(nc, in_maps, list(range(N_CORES)))
    full = np.concatenate([res.results[i]["out"] for i in range(N_CORES)], axis=0)
    return full.reshape(B, N, H, W).astype(np.float32)


# revision 11
# speedup vs baseline: 2.3675x; 1.1953x over previous
"""Trainium2 Bass kernel for nn_CropAndPadMask (paste instance masks into canvases).

Math: for each (b, n) box the reference output is a bilinear resize of the
28x28 mask pasted into a zero [H, W] canvas.  Bilinear resize + paste is a
pair of small matmuls:

    out[b, n] = Wy[b, n] @ mask[b, n] @ Wx[b, n]

with Wy [H, 28] holding the y-interpolation weights (box/validity mask folded
in) and Wx [28, W] the x-interpolation weights.  The host precomputes these
tiny weight matrices from det_outs; the device does the heavy part: 5 matmuls
per canvas on TensorE and streams the 256 MiB of canvases out to HBM.

Sharding: 256 canvases are split 32-per-core across the 8 NeuronCores
(pure data parallel, no communication).
"""

import sys

for _p in ("/opt/trn_rl_repo", "/root/.axon_site/_ro/trn_rl_repo"):
    if _p not in sys.path:
        sys.path.append(_p)

import numpy as np

import concourse.bass as bass
import concourse.mybir as mybir
import concourse.tile as tile
from concourse.bass_utils import run_bass_kernel_spmd

B, N, H, W, MH, MW = 4, 64, 512, 512, 28, 28
N_CORES = 8
CPC = (B * N) // N_CORES  # canvases per core = 32
G = 2                     # canvases per output DMA group
KCH = 4                   # row chunks per canvas (H = KCH * 128)
FREE = 2 * W + MW         # per-canvas weight row: wyT | wx | maskT

def _split_multi_waits(nc: bass.Bass) -> None:
    """The walrus bundled in this container accepts at most ONE sync wait per
    instruction.  Tile freely attaches several.  Hoist the extras onto
    standalone EventSemaphore carriers inserted just before the instruction on
    the same engine (per-engine program order makes this equivalent)."""
    n_new = 0
    for f in nc.m.functions:
        for bb in f.blocks:
            lst = bb.instructions
            i = 0
            while i < len(lst):
                ins = lst[i]
                si = ins.sync_info
                if si is not None and si.on_wait and len(si.on_wait) > 1:
                    waits = list(si.on_wait)
                    ins.sync_info = mybir.SyncInfo(
                        on_wait=waits[:1], on_update=list(si.on_update or [])
                    )
                    carriers = []
                    for w in waits[1:]:
                        n_new += 1
                        carriers.append(
                            mybir.InstEventSemaphore(
                                name=f"I-waitsplit-{n_new}",
                                ins=[],
                                outs=[],
                                engine=ins.engine,
                                sync_info=mybir.SyncInfo(on_wait=[w], on_update=[]),
                            )
                        )
                    lst[i:i] = carriers
                    i += len(carriers)
                i += 1


def build_nc(cpc: int = CPC, g: int = G) -> bass.Bass:
    """One core's program: cpc canvases, streamed out g at a time."""
    f32 = mybir.dt.float32
    f32r = mybir.dt.float32r
    nc = bass.Bass()
    wmat = nc.dram_tensor("wmat", [cpc, MW, FREE], f32r, kind="ExternalInput")
    out = nc.dram_tensor("out", [cpc, H, W], f32, kind="ExternalOutput")

    PREFETCH = 3  # input DMAs issued this many canvases ahead

    with tile.TileContext(nc) as tc:
        with (
            tc.tile_pool(name="win", bufs=PREFETCH + 3) as win_pool,
            tc.tile_pool(name="ssb", bufs=3) as s_pool,
            tc.tile_pool(name="stage", bufs=4) as stage_pool,
            tc.tile_pool(name="psum_s", bufs=2, space="PSUM") as psum_s,
            tc.tile_pool(name="psum_c", bufs=6, space="PSUM") as psum_c,
        ):
            w_tiles: dict[int, object] = {}
            s_tiles: dict[int, object] = {}
            n_cp = 0

            def load_w(c):
                if c < cpc and c not in w_tiles:
                    w_t = win_pool.tile([MW, FREE], f32r)
                    nc.gpsimd.dma_start(w_t[:], wmat[c])
                    w_tiles[c] = w_t

            def mm1_and_scopy(c):
                """S = mask @ Wx : lhsT = mask^T [MW, MH], rhs = Wx [MW, W]"""
                nonlocal n_cp
                if c >= cpc or c in s_tiles:
                    return
                w_t = w_tiles[c]
                s_p = psum_s.tile([MH, W], f32)
                nc.tensor.matmul(
                    s_p[:], w_t[:, 2 * W :], w_t[:, W : 2 * W],
                    start=True, stop=True,
                )
                s_sb = s_pool.tile([MH, W], f32r)
                if n_cp % 2 == 0:
                    nc.scalar.copy(s_sb[:], s_p[:])
                else:
                    nc.vector.tensor_copy(s_sb[:], s_p[:])
                n_cp += 1
                s_tiles[c] = s_sb

            for c in range(min(PREFETCH, cpc)):
                load_w(c)
            mm1_and_scopy(0)

            for grp in range(cpc // g):
                stage = stage_pool.tile([128, g, KCH, W], f32)
                for cc in range(g):
                    c = grp * g + cc
                    load_w(c + PREFETCH)
                    # pipeline: next canvas's S is produced while this one's
                    # chunk matmuls stream, so PE never waits on the S-copy
                    mm1_and_scopy(c + 1)
                    w_t, s_sb = w_tiles.pop(c), s_tiles.pop(c)
                    for k in range(KCH):
                        # rows {k, k+4, ...} of the canvas: lhsT = WyT cols k::4
                        p_k = psum_c.tile([128, W], f32)
                        nc.tensor.matmul(
                            p_k[:], w_t[:, 128 * k : 128 * (k + 1)], s_sb[:],
                            start=True, stop=True,
                        )
                        if n_cp % 2 == 0:
                            nc.scalar.copy(stage[:, cc, k, :], p_k[:])
                        else:
                            nc.vector.tensor_copy(stage[:, cc, k, :], p_k[:])
                        n_cp += 1
                # canvas row = 4*p + k  ->  DRAM view [p, c, k, w]
                out_ap = out[grp * g : (grp + 1) * g].rearrange(
                    "c (p k) w -> p c k w", k=KCH
                )
                nc.sync.dma_start(out_ap, stage[:])
    _split_multi_waits(nc)
    return nc


def _box_weight_matrices(det_outs: np.ndarray) -> tuple[np.ndarray, np.ndarray]:
    """Wy [BN, H, MH] and Wx [BN, MW, W] (f32), reference semantics."""
    det = np.asarray(det_outs, dtype=np.float32).reshape(B * N, 6)
    score = det[:, 5]
    thr = np.float32(50.0) if np.max(score) > 50.0 else np.float32(-100.0)
    valid = score >= thr
    box = np.maximum(det, np.float32(1.0))
    cx, cy, w, h = box[:, 0], box[:, 1], box[:, 2], box[:, 3]
    two = np.float32(2.0)
    xmin = np.clip(np.ceil(cx - w / two).astype(np.int32), 0, W)
    xmax = np.clip(np.ceil(cx + w / two).astype(np.int32), 0, W)
    ymin = np.clip(np.ceil(cy - h / two).astype(np.int32), 0, H)
    ymax = np.clip(np.ceil(cy + h / two).astype(np.int32), 0, H)
    out_h = (ymax - ymin).astype(np.float32)
    out_w = (xmax - xmin).astype(np.float32)
    one = np.float32(1.0)
    sy = np.where(out_h > one, np.float32(MH - 1) / np.maximum(out_h - one, one),
                  np.float32(0.0)).astype(np.float32)
    sx = np.where(out_w > one, np.float32(MW - 1) / np.maximum(out_w - one, one),
                  np.float32(0.0)).astype(np.float32)

    ys = np.arange(H, dtype=np.float32)
    xs = np.arange(W, dtype=np.float32)
    src_y = (ys[None, :] - ymin[:, None].astype(np.float32)) * sy[:, None]
    src_x = (xs[None, :] - xmin[:, None].astype(np.float32)) * sx[:, None]
    src_y = np.clip(src_y, np.float32(0.0), np.float32(MH - 1)).astype(np.float32)
    src_x = np.clip(src_x, np.float32(0.0), np.float32(MW - 1)).astype(np.float32)

    y0 = np.floor(src_y).astype(np.int32)
    y1 = np.minimum(y0 + 1, MH - 1)
    wy = (src_y - y0.astype(np.float32)).astype(np.float32)
    x0 = np.floor(src_x).astype(np.int32)
    x1 = np.minimum(x0 + 1, MW - 1)
    wx = (src_x - x0.astype(np.float32)).astype(np.float32)

    keep_y = ((ys[None, :] >= ymin[:, None].astype(np.float32))
              & (ys[None, :] < ymax[:, None].astype(np.float32))
              & valid[:, None]).astype(np.float32)
    keep_x = ((xs[None, :] >= xmin[:, None].astype(np.float32))
              & (xs[None, :] < xmax[:, None].astype(np.float32))).astype(np.float32)

    m = np.arange(MH, dtype=np.int32)
    Wy = ((m[None, None, :] == y0[:, :, None]) * (one - wy[:, :, None])
          + (m[None, None, :] == y1[:, :, None]) * wy[:, :, None]).astype(np.float32)
    Wy *= keep_y[:, :, None]
    Wx = ((m[None, :, None] == x0[:, None, :]) * (one - wx[:, None, :])
          + (m[None, :, None] == x1[:, None, :]) * wx[:, None, :]).astype(np.float32)
    Wx *= keep_x[:, None, :]
    return Wy, Wx


_ROW_PERM = np.concatenate([KCH * np.arange(H // KCH) + k for k in range(KCH)])


def prepare_in_maps(det_outs: np.ndarray, ins_outs: np.ndarray,
                    cpc: int = CPC, n_cores: int = N_CORES) -> list[dict]:
    Wy, Wx = _box_weight_matrices(det_outs)
    # wyT [BN, MH, H], columns permuted so block k holds rows k::KCH
    wyT = np.ascontiguousarray(np.transpose(Wy, (0, 2, 1)))[:, :, _ROW_PERM]
    masksT = np.ascontiguousarray(
        np.transpose(np.asarray(ins_outs, np.float32).reshape(B * N, MH, MW),
                     (0, 2, 1)))
    wmat = np.concatenate([wyT, Wx, masksT], axis=2).astype(np.float32)
    assert wmat.shape == (B * N, MW, FREE)
    return [{"wmat": np.ascontiguousarray(wmat[i * cpc : (i + 1) * cpc])}
            for i in range(n_cores)]


def kernel(images: np.ndarray, det_outs: np.ndarray, ins_outs: np.ndarray) -> np.ndarray:
    nc = build_nc()
    in_maps = prepare_in_maps(det_outs, ins_outs)
    res = run_bass_kernel_spmd(nc, in_maps, list(range(N_CORES)))
    full = np.concatenate([res.results[i]["out"] for i in range(N_CORES)], axis=0)
    return full.reshape(B, N, H, W).astype(np.float32)


# revision 13
# speedup vs baseline: 2.3753x; 1.0033x over previous
"""Trainium2 Bass kernel for nn_CropAndPadMask (paste instance masks into canvases).

Math: for each (b, n) box the reference output is a bilinear resize of the
28x28 mask pasted into a zero [H, W] canvas.  Bilinear resize + paste is a
pair of small matmuls:

    out[b, n] = Wy[b, n] @ mask[b, n] @ Wx[b, n]

with Wy [H, 28] holding the y-interpolation weights (box/validity mask folded
in) and Wx [28, W] the x-interpolation weights.  The host precomputes these
tiny weight matrices from det_outs; the device does the heavy part: 5 matmuls
per canvas on TensorE and streams the 256 MiB of canvases out to HBM.

Sharding: 256 canvases are split 32-per-core across the 8 NeuronCores
(pure data parallel, no communication).
"""

import sys

for _p in ("/opt/trn_rl_repo", "/root/.axon_site/_ro/trn_rl_repo"):
    if _p not in sys.path:
        sys.path.append(_p)

import numpy as np

import concourse.bass as bass
import concourse.mybir as mybir
import concourse.tile as tile
from concourse.bass_utils import run_bass_kernel_spmd

B, N, H, W, MH, MW = 4, 64, 512, 512, 28, 28
N_CORES = 8
CPC = (B * N) // N_CORES  # canvases per core = 32
G = 2                     # canvases per output DMA group
KCH = 4                   # row chunks per canvas (H = KCH * 128)
FREE = 2 * W + MW         # per-canvas weight row: wyT | wx | maskT

def _split_multi_waits(nc: bass.Bass) -> None:
    """The walrus bundled in this container accepts at most ONE sync wait per
    instruction.  Tile freely attaches several.  Hoist the extras onto
    standalone EventSemaphore carriers inserted just before the instruction on
    the same engine (per-engine program order makes this equivalent)."""
    n_new = 0
    for f in nc.m.functions:
        for bb in f.blocks:
            lst = bb.instructions
            i = 0
            while i < len(lst):
                ins = lst[i]
                si = ins.sync_info
                if si is not None and si.on_wait and len(si.on_wait) > 1:
                    waits = list(si.on_wait)
                    ins.sync_info = mybir.SyncInfo(
                        on_wait=waits[:1], on_update=list(si.on_update or [])
                    )
                    carriers = []
                    for w in waits[1:]:
                        n_new += 1
                        carriers.append(
                            mybir.InstEventSemaphore(
                                name=f"I-waitsplit-{n_new}",
                                ins=[],
                                outs=[],
                                engine=ins.engine,
                                sync_info=mybir.SyncInfo(on_wait=[w], on_update=[]),
                            )
                        )
                    lst[i:i] = carriers
                    i += len(carriers)
                i += 1


def build_nc(cpc: int = CPC, g: int = G) -> bass.Bass:
    """One core's program: cpc canvases, streamed out g at a time."""
    f32 = mybir.dt.float32
    f32r = mybir.dt.float32r
    nc = bass.Bass()
    wmat = nc.dram_tensor("wmat", [cpc, MW, FREE], f32r, kind="ExternalInput")
    out = nc.dram_tensor("out", [cpc, H, W], f32, kind="ExternalOutput")

    PREFETCH = 3  # input DMAs issued this many canvases ahead

    with tile.TileContext(nc) as tc:
        with (
            tc.tile_pool(name="win", bufs=PREFETCH + 3) as win_pool,
            tc.tile_pool(name="ssb", bufs=3) as s_pool,
            tc.tile_pool(name="stage", bufs=4) as stage_pool,
            tc.tile_pool(name="psum_s", bufs=2, space="PSUM") as psum_s,
            tc.tile_pool(name="psum_c", bufs=3, space="PSUM") as psum_c,
        ):
            w_tiles: dict[int, object] = {}
            s_tiles: dict[int, object] = {}
            n_cp = 0

            def load_w(c):
                if c < cpc and c not in w_tiles:
                    w_t = win_pool.tile([MW, FREE], f32r)
                    nc.gpsimd.dma_start(w_t[:], wmat[c])
                    w_tiles[c] = w_t

            def mm1_and_scopy(c):
                """S = mask @ Wx : lhsT = mask^T [MW, MH], rhs = Wx [MW, W]"""
                nonlocal n_cp
                if c >= cpc or c in s_tiles:
                    return
                w_t = w_tiles[c]
                s_p = psum_s.tile([MH, W], f32)
                nc.tensor.matmul(
                    s_p[:], w_t[:, 2 * W :], w_t[:, W : 2 * W],
                    start=True, stop=True,
                )
                s_sb = s_pool.tile([MH, W], f32r)
                if n_cp % 2 == 0:
                    nc.scalar.copy(s_sb[:], s_p[:])
                else:
                    nc.vector.tensor_copy(s_sb[:], s_p[:])
                n_cp += 1
                s_tiles[c] = s_sb

            for c in range(min(PREFETCH, cpc)):
                load_w(c)
            mm1_and_scopy(0)

            # small first/last groups: the first out-DMA starts after one
            # canvas (shorter pipeline fill) and the final drain is 1 MiB
            if cpc >= 8:
                group_sizes = [1, 1] + [g] * ((cpc - 4) // g) + [1, 1]
            else:
                group_sizes = [g] * (cpc // g)
            assert sum(group_sizes) == cpc
            c0 = 0
            for gsz in group_sizes:
                stage = stage_pool.tile([128, g, KCH, W], f32, tag="stage")
                for cc in range(gsz):
                    c = c0 + cc
                    load_w(c + PREFETCH)
                    # pipeline: next canvas's S is produced while this one's
                    # chunk matmuls stream, so PE never waits on the S-copy
                    mm1_and_scopy(c + 1)
                    w_t, s_sb = w_tiles.pop(c), s_tiles.pop(c)
                    for k in range(0, KCH, 2):
                        # rows {k, k+4, ...} of the canvas: lhsT = WyT cols
                        # k::4; two chunks share a 2-bank PSUM tile so one
                        # copy moves both
                        p_k = psum_c.tile([128, 2, W], f32)
                        nc.tensor.matmul(
                            p_k[:, 0, :], w_t[:, 128 * k : 128 * (k + 1)],
                            s_sb[:], start=True, stop=True,
                        )
                        nc.tensor.matmul(
                            p_k[:, 1, :], w_t[:, 128 * (k + 1) : 128 * (k + 2)],
                            s_sb[:], start=True, stop=True,
                        )
                        if n_cp % 2 == 0:
                            nc.scalar.copy(stage[:, cc, k : k + 2, :], p_k[:])
                        else:
                            nc.vector.tensor_copy(stage[:, cc, k : k + 2, :], p_k[:])
                        n_cp += 1
                # canvas row = 4*p + k  ->  DRAM view [p, c, k, w]
                out_ap = out[c0 : c0 + gsz].rearrange("c (p k) w -> p c k w", k=KCH)
                nc.sync.dma_start(out_ap, stage[:, :gsz])
                c0 += gsz
    _split_multi_waits(nc)
    return nc


def _box_weight_matrices(det_outs: np.ndarray) -> tuple[np.ndarray, np.ndarray]:
    """Wy [BN, H, MH] and Wx [BN, MW, W] (f32), reference semantics."""
    det = np.asarray(det_outs, dtype=np.float32).reshape(B * N, 6)
    score = det[:, 5]
    thr = np.float32(50.0) if np.max(score) > 50.0 else np.float32(-100.0)
    valid = score >= thr
    box = np.maximum(det, np.float32(1.0))
    cx, cy, w, h = box[:, 0], box[:, 1], box[:, 2], box[:, 3]
    two = np.float32(2.0)
    xmin = np.clip(np.ceil(cx - w / two).astype(np.int32), 0, W)
    xmax = np.clip(np.ceil(cx + w / two).astype(np.int32), 0, W)
    ymin = np.clip(np.ceil(cy - h / two).astype(np.int32), 0, H)
    ymax = np.clip(np.ceil(cy + h / two).astype(np.int32), 0, H)
    out_h = (ymax - ymin).astype(np.float32)
    out_w = (xmax - xmin).astype(np.float32)
    one = np.float32(1.0)
    sy = np.where(out_h > one, np.float32(MH - 1) / np.maximum(out_h - one, one),
                  np.float32(0.0)).astype(np.float32)
    sx = np.where(out_w > one, np.float32(MW - 1) / np.maximum(out_w - one, one),
                  np.float32(0.0)).astype(np.float32)

    ys = np.arange(H, dtype=np.float32)
    xs = np.arange(W, dtype=np.float32)
    src_y = (ys[None, :] - ymin[:, None].astype(np.float32)) * sy[:, None]
    src_x = (xs[None, :] - xmin[:, None].astype(np.float32)) * sx[:, None]
    src_y = np.clip(src_y, np.float32(0.0), np.float32(MH - 1)).astype(np.float32)
    src_x = np.clip(src_x, np.float32(0.0), np.float32(MW - 1)).astype(np.float32)

    y0 = np.floor(src_y).astype(np.int32)
    y1 = np.minimum(y0 + 1, MH - 1)
    wy = (src_y - y0.astype(np.float32)).astype(np.float32)
    x0 = np.floor(src_x).astype(np.int32)
    x1 = np.minimum(x0 + 1, MW - 1)
    wx = (src_x - x0.astype(np.float32)).astype(np.float32)

    keep_y = ((ys[None, :] >= ymin[:, None].astype(np.float32))
              & (ys[None, :] < ymax[:, None].astype(np.float32))
              & valid[:, None]).astype(np.float32)
    keep_x = ((xs[None, :] >= xmin[:, None].astype(np.float32))
              & (xs[None, :] < xmax[:, None].astype(np.float32))).astype(np.float32)

    m = np.arange(MH, dtype=np.int32)
    Wy = ((m[None, None, :] == y0[:, :, None]) * (one - wy[:, :, None])
          + (m[None, None, :] == y1[:, :, None]) * wy[:, :, None]).astype(np.float32)
    Wy *= keep_y[:, :, None]
    Wx = ((m[None, :, None] == x0[:, None, :]) * (one - wx[:, None, :])
          + (m[None, :, None] == x1[:, None, :]) * wx[:, None, :]).astype(np.float32)
    Wx *= keep_x[:, None, :]
    return Wy, Wx


_ROW_PERM = np.concatenate([KCH * np.arange(H // KCH) + k for k in range(KCH)])


def prepare_in_maps(det_outs: np.ndarray, ins_outs: np.ndarray,
                    cpc: int = CPC, n_cores: int = N_CORES) -> list[dict]:
    Wy, Wx = _box_weight_matrices(det_outs)
    # wyT [BN, MH, H], columns permuted so block k holds rows k::KCH
    wyT = np.ascontiguousarray(np.transpose(Wy, (0, 2, 1)))[:, :, _ROW_PERM]
    masksT = np.ascontiguousarray(
        np.transpose(np.asarray(ins_outs, np.float32).reshape(B * N, MH, MW),
                     (0, 2, 1)))
    wmat = np.concatenate([wyT, Wx, masksT], axis=2).astype(np.float32)
    assert wmat.shape == (B * N, MW, FREE)
    return [{"wmat": np.ascontiguousarray(wmat[i * cpc : (i + 1) * cpc])}
            for i in range(n_cores)]


def kernel(images: np.ndarray, det_outs: np.ndarray, ins_outs: np.ndarray) -> np.ndarray:
    nc = build_nc()
    in_maps = prepare_in_maps(det_outs, ins_outs)
    res = run_bass_kernel_spmd(nc, in_maps, list(range(N_CORES)))
    full = np.concatenate([res.results[i]["out"] for i in range(N_CORES)], axis=0)
    return full.reshape(B, N, H, W).astype(np.float32)


# revision 15
# speedup vs baseline: 2.3789x; 1.0015x over previous
"""Trainium2 Bass kernel for nn_CropAndPadMask (paste instance masks into canvases).

Math: for each (b, n) box the reference output is a bilinear resize of the
28x28 mask pasted into a zero [H, W] canvas.  Bilinear resize + paste is a
pair of small matmuls:

    out[b, n] = Wy[b, n] @ mask[b, n] @ Wx[b, n]

with Wy [H, 28] holding the y-interpolation weights (box/validity mask folded
in) and Wx [28, W] the x-interpolation weights.  The host precomputes these
tiny weight matrices from det_outs; the device does the heavy part: 5 matmuls
per canvas on TensorE and streams the 256 MiB of canvases out to HBM.

Sharding: 256 canvases are split 32-per-core across the 8 NeuronCores
(pure data parallel, no communication).
"""

import sys

for _p in ("/opt/trn_rl_repo", "/root/.axon_site/_ro/trn_rl_repo"):
    if _p not in sys.path:
        sys.path.append(_p)

import numpy as np

import concourse.bass as bass
import concourse.mybir as mybir
import concourse.tile as tile
from concourse.bass_utils import run_bass_kernel_spmd

B, N, H, W, MH, MW = 4, 64, 512, 512, 28, 28
N_CORES = 8
CPC = (B * N) // N_CORES  # canvases per core = 32
G = 2                     # canvases per output DMA group
KCH = 4                   # row chunks per canvas (H = KCH * 128)
FREE = 2 * W + MW         # per-canvas weight row: wyT | wx | maskT

def _split_multi_waits(nc: bass.Bass) -> None:
    """The walrus bundled in this container accepts at most ONE sync wait per
    instruction.  Tile freely attaches several.  Hoist the extras onto
    standalone EventSemaphore carriers inserted just before the instruction on
    the same engine (per-engine program order makes this equivalent)."""
    n_new = 0
    for f in nc.m.functions:
        for bb in f.blocks:
            lst = bb.instructions
            i = 0
            while i < len(lst):
                ins = lst[i]
                si = ins.sync_info
                if si is not None and si.on_wait and len(si.on_wait) > 1:
                    waits = list(si.on_wait)
                    ins.sync_info = mybir.SyncInfo(
                        on_wait=waits[:1], on_update=list(si.on_update or [])
                    )
                    carriers = []
                    for w in waits[1:]:
                        n_new += 1
                        carriers.append(
                            mybir.InstEventSemaphore(
                                name=f"I-waitsplit-{n_new}",
                                ins=[],
                                outs=[],
                                engine=ins.engine,
                                sync_info=mybir.SyncInfo(on_wait=[w], on_update=[]),
                            )
                        )
                    lst[i:i] = carriers
                    i += len(carriers)
                i += 1


def build_nc(cpc: int = CPC, g: int = G) -> bass.Bass:
    """One core's program: cpc canvases, streamed out g at a time."""
    f32 = mybir.dt.float32
    f32r = mybir.dt.float32r
    nc = bass.Bass()
    wmat = nc.dram_tensor("wmat", [cpc, MW, FREE], f32r, kind="ExternalInput")
    out = nc.dram_tensor("out", [cpc, H, W], f32, kind="ExternalOutput")

    PREFETCH = 4  # input DMAs issued this many canvases ahead

    with tile.TileContext(nc) as tc:
        with (
            tc.tile_pool(name="win", bufs=PREFETCH + 3) as win_pool,
            tc.tile_pool(name="ssb", bufs=4) as s_pool,
            tc.tile_pool(name="stage", bufs=4) as stage_pool,
            tc.tile_pool(name="psum_s", bufs=2, space="PSUM") as psum_s,
            tc.tile_pool(name="psum_c", bufs=6, space="PSUM") as psum_c,
        ):
            w_tiles: dict[int, object] = {}
            s_tiles: dict[int, object] = {}
            n_cp = 0

            def load_w(c):
                if c < cpc and c not in w_tiles:
                    w_t = win_pool.tile([MW, FREE], f32r)
                    nc.gpsimd.dma_start(w_t[:], wmat[c])
                    w_tiles[c] = w_t

            def mm1_and_scopy(c):
                """S = mask @ Wx : lhsT = mask^T [MW, MH], rhs = Wx [MW, W]"""
                nonlocal n_cp
                if c >= cpc or c in s_tiles:
                    return
                w_t = w_tiles[c]
                s_p = psum_s.tile([MH, W], f32)
                nc.tensor.matmul(
                    s_p[:], w_t[:, 2 * W :], w_t[:, W : 2 * W],
                    start=True, stop=True,
                )
                s_sb = s_pool.tile([MH, W], f32r)
                if n_cp % 2 == 0:
                    nc.scalar.copy(s_sb[:], s_p[:])
                else:
                    nc.vector.tensor_copy(s_sb[:], s_p[:])
                n_cp += 1
                s_tiles[c] = s_sb

            for c in range(min(PREFETCH, cpc)):
                load_w(c)
            mm1_and_scopy(0)

            # small first/last groups: the first out-DMA starts after one
            # canvas (shorter pipeline fill) and the final drain is 1 MiB
            if cpc >= 8:
                group_sizes = [1, 1] + [g] * ((cpc - 4) // g) + [1, 1]
            else:
                group_sizes = [g] * (cpc // g)
            assert sum(group_sizes) == cpc
            c0 = 0
            for gsz in group_sizes:
                stage = stage_pool.tile([128, g, KCH, W], f32, tag="stage")
                for cc in range(gsz):
                    c = c0 + cc
                    load_w(c + PREFETCH)
                    # pipeline: next canvas's S is produced while this one's
                    # chunk matmuls stream, so PE never waits on the S-copy
                    mm1_and_scopy(c + 1)
                    w_t, s_sb = w_tiles.pop(c), s_tiles.pop(c)
                    for k in range(KCH):
                        # rows {k, k+4, ...} of the canvas: lhsT = WyT cols k::4
                        p_k = psum_c.tile([128, W], f32)
                        nc.tensor.matmul(
                            p_k[:], w_t[:, 128 * k : 128 * (k + 1)], s_sb[:],
                            start=True, stop=True,
                        )
                        if n_cp % 2 == 0:
                            nc.scalar.copy(stage[:, cc, k, :], p_k[:])
                        else:
                            nc.vector.tensor_copy(stage[:, cc, k, :], p_k[:])
                        n_cp += 1
                # canvas row = 4*p + k  ->  DRAM view [p, c, k, w]
                out_ap = out[c0 : c0 + gsz].rearrange("c (p k) w -> p c k w", k=KCH)
                nc.sync.dma_start(out_ap, stage[:, :gsz])
                c0 += gsz
    _split_multi_waits(nc)
    return nc


def _box_weight_matrices(det_outs: np.ndarray) -> tuple[np.ndarray, np.ndarray]:
    """Wy [BN, H, MH] and Wx [BN, MW, W] (f32), reference semantics."""
    det = np.asarray(det_outs, dtype=np.float32).reshape(B * N, 6)
    score = det[:, 5]
    thr = np.float32(50.0) if np.max(score) > 50.0 else np.float32(-100.0)
    valid = score >= thr
    box = np.maximum(det, np.float32(1.0))
    cx, cy, w, h = box[:, 0], box[:, 1], box[:, 2], box[:, 3]
    two = np.float32(2.0)
    xmin = np.clip(np.ceil(cx - w / two).astype(np.int32), 0, W)
    xmax = np.clip(np.ceil(cx + w / two).astype(np.int32), 0, W)
    ymin = np.clip(np.ceil(cy - h / two).astype(np.int32), 0, H)
    ymax = np.clip(np.ceil(cy + h / two).astype(np.int32), 0, H)
    out_h = (ymax - ymin).astype(np.float32)
    out_w = (xmax - xmin).astype(np.float32)
    one = np.float32(1.0)
    sy = np.where(out_h > one, np.float32(MH - 1) / np.maximum(out_h - one, one),
                  np.float32(0.0)).astype(np.float32)
    sx = np.where(out_w > one, np.float32(MW - 1) / np.maximum(out_w - one, one),
                  np.float32(0.0)).astype(np.float32)

    ys = np.arange(H, dtype=np.float32)
    xs = np.arange(W, dtype=np.float32)
    src_y = (ys[None, :] - ymin[:, None].astype(np.float32)) * sy[:, None]
    src_x = (xs[None, :] - xmin[:, None].astype(np.float32)) * sx[:, None]
    src_y = np.clip(src_y, np.float32(0.0), np.float32(MH - 1)).astype(np.float32)
    src_x = np.clip(src_x, np.float32(0.0), np.float32(MW - 1)).astype(np.float32)

    y0 = np.floor(src_y).astype(np.int32)
    y1 = np.minimum(y0 + 1, MH - 1)
    wy = (src_y - y0.astype(np.float32)).astype(np.float32)
    x0 = np.floor(src_x).astype(np.int32)
    x1 = np.minimum(x0 + 1, MW - 1)
    wx = (src_x - x0.astype(np.float32)).astype(np.float32)

    keep_y = ((ys[None, :] >= ymin[:, None].astype(np.float32))
              & (ys[None, :] < ymax[:, None].astype(np.float32))
              & valid[:, None]).astype(np.float32)
    keep_x = ((xs[None, :] >= xmin[:, None].astype(np.float32))
              & (xs[None, :] < xmax[:, None].astype(np.float32))).astype(np.float32)

    m = np.arange(MH, dtype=np.int32)
    Wy = ((m[None, None, :] == y0[:, :, None]) * (one - wy[:, :, None])
          + (m[None, None, :] == y1[:, :, None]) * wy[:, :, None]).astype(np.float32)
    Wy *= keep_y[:, :, None]
    Wx = ((m[None, :, None] == x0[:, None, :]) * (one - wx[:, None, :])
          + (m[None, :, None] == x1[:, None, :]) * wx[:, None, :]).astype(np.float32)
    Wx *= keep_x[:, None, :]
    return Wy, Wx


_ROW_PERM = np.concatenate([KCH * np.arange(H // KCH) + k for k in range(KCH)])


def prepare_in_maps(det_outs: np.ndarray, ins_outs: np.ndarray,
                    cpc: int = CPC, n_cores: int = N_CORES) -> list[dict]:
    Wy, Wx = _box_weight_matrices(det_outs)
    # wyT [BN, MH, H], columns permuted so block k holds rows k::KCH
    wyT = np.ascontiguousarray(np.transpose(Wy, (0, 2, 1)))[:, :, _ROW_PERM]
    masksT = np.ascontiguousarray(
        np.transpose(np.asarray(ins_outs, np.float32).reshape(B * N, MH, MW),
                     (0, 2, 1)))
    wmat = np.concatenate([wyT, Wx, masksT], axis=2).astype(np.float32)
    assert wmat.shape == (B * N, MW, FREE)
    return [{"wmat": np.ascontiguousarray(wmat[i * cpc : (i + 1) * cpc])}
            for i in range(n_cores)]


def kernel(images: np.ndarray, det_outs: np.ndarray, ins_outs: np.ndarray) -> np.ndarray:
    nc = build_nc()
    in_maps = prepare_in_maps(det_outs, ins_outs)
    res = run_bass_kernel_spmd(nc, in_maps, list(range(N_CORES)))
    full = np.concatenate([res.results[i]["out"] for i in range(N_CORES)], axis=0)
    return full.reshape(B, N, H, W).astype(np.float32)
